# revision 1
# baseline (speedup 1.0000x reference)
"""AutoCorrelation (Autoformer-style) Bass kernel for Trainium2, 8 NeuronCores.

Full inputs in, full outputs out. Data-parallel over batch: B=16 -> 2 batches
per core. Per core, per batch:

  A. Channel projections on the PE:
       qT[t,d] = sum_c query[c,t] Wq[c,d]   (t on partitions -> feeds DFT)
       kT[t,d] likewise
       v[d,t]  = sum_c Wv[c,d] value[c,t]   (d on partitions -> row gather)
     v is written twice side-by-side into DRAM table v2[b*512+d, 4096] for
     circular-shift gathers.
  B. Forward real DFT via matmuls with cos/-sin matrices (F padded to 1152),
     fused pointwise P = FQ * conj(FK) on the DVE.
  C. Inverse DFT via matmuls: r[c,t] = sum_f Pre*ci + Pim*sn, where ci/sn
     carry the irfft weights (1,2,...,2,1)/T^2 and sn = -w*sin/T^2.
  D. Per 128-channel tile: top-8 values+indices (InstMax/InstMaxIndex),
     softmax weights of the top-3 straight from the top values
     (w_k = exp(v_k - v_0) / sum_t exp(r - v_0)), circular-shift rows of v
     via indirect-DMA gather with flat element offsets into agg[k*C+c, t],
     scaled by w_k.
  E. out[d,t] = sum_e Wf[e,d] agg[e,t] via 12-chunk PE accumulation.

All matmuls run in fp32: the top-3 lag selection needs auto_corr to match the
reference's fp32 FFT to ~1e-7 absolute; fp32 PE matmuls measure ~2.4e-7
relative which lands flips at the same rate as the reference's own fp32 error.
bf16/tf32-grade products would scramble the top-3 ordering entirely.

Biases are all zero in this problem's setup_inputs(); asserted host-side.
"""
import numpy as np

import concourse.bass as bass
import concourse.tile as tile
from concourse import bacc, mybir

dt = mybir.dt
AF = mybir.ActivationFunctionType
OP = mybir.AluOpType

P = 128
B, C, T, K = 16, 512, 2048, 3
NB = 2                    # batches per core
NCORES = 8
F = 1152                  # rfft bins 1025, padded to 9*128
TC = T // P               # 16 t-chunks
CC = C // P               # 4 c-chunks
FC = F // P               # 9 f-chunks
NE = K * C // P           # 12 e-chunks of Wf / agg
TE = 1152                 # even-part length 1025, padded to 9*128
TEC = TE // P             # 9
TO = 1024                 # odd-part length (t=1..1023 live)
TOC = TO // P             # 8
H = T // 2                # 1024

_CACHE = {}


def _dft_matrices():
    """Radix-split DFT matrices (fp64 -> fp32).

    Level-1 even/odd in t (qe/qo), then level-2 split by f parity:
      FQre over even f contracts xee (t=0..512), odd f contracts xeo (t=0..511)
      FQim over even f contracts xoo (t=1..511), odd f contracts xoe (t=1..512)
    Frequency storage is parity-permuted: chunks [0:5]=even f (2g, g<=512),
    chunks [5:9]=odd f (2g+1). Inverse matrices have rows permuted to match.
    """
    t640 = np.arange(640.0)[:, None]
    t512 = np.arange(512.0)[:, None]
    ge = np.arange(640.0)[None, :]
    go = np.arange(512.0)[None, :]
    wree = np.where((t640 <= 512) & (ge <= 512),
                    np.cos(2 * np.pi * t640 * (2 * ge) / T), 0.0).astype(np.float32)
    wreo = np.cos(2 * np.pi * t512 * (2 * go + 1) / T).astype(np.float32)
    wime = np.where(ge <= 512,
                    -np.sin(2 * np.pi * t512 * (2 * ge) / T), 0.0).astype(np.float32)
    wimo = np.where(t640 <= 512,
                    -np.sin(2 * np.pi * t640 * (2 * go + 1) / T), 0.0).astype(np.float32)

    f64 = np.arange(F, dtype=np.float64)[None, :]
    livef = f64 <= H
    w = np.where((f64 == 0) | (f64 == H), 1.0, 2.0) * livef / (T * T)
    fc_ = f64.T
    tt = np.arange(TE, dtype=np.float64)[None, :]
    cie = np.where((fc_ <= H) & (tt <= H),
                   np.cos(2 * np.pi * fc_ * tt / T) * w.T, 0.0)
    tt2 = np.arange(TO, dtype=np.float64)[None, :]
    sie = np.where(fc_ <= H,
                   -np.sin(2 * np.pi * fc_ * tt2 / T) * w.T, 0.0)

    def permrows(m):
        out = np.zeros_like(m)
        out[0:513] = m[0:1025:2]
        out[640:1152] = m[1:1024:2]
        return out

    return (wree, wreo, wime, wimo,
            permrows(cie).astype(np.float32), permrows(sie).astype(np.float32))


def _build():
    nc = bacc.Bacc("TRN2", target_bir_lowering=False, debug=False,
                   num_devices=NCORES)

    query2 = nc.dram_tensor("query2", [NB, C, T], dt.float32, kind="ExternalInput").ap()
    key2 = nc.dram_tensor("key2", [NB, C, T], dt.float32, kind="ExternalInput").ap()
    value2 = nc.dram_tensor("value2", [NB, C, T], dt.float32, kind="ExternalInput").ap()
    Wq = nc.dram_tensor("Wq", [C, C], dt.float32, kind="ExternalInput").ap()
    Wk = nc.dram_tensor("Wk", [C, C], dt.float32, kind="ExternalInput").ap()
    Wv = nc.dram_tensor("Wv", [C, C], dt.float32, kind="ExternalInput").ap()
    Wf = nc.dram_tensor("Wf", [K * C, C], dt.float32, kind="ExternalInput").ap()
    Wree = nc.dram_tensor("Wree", [640, 640], dt.float32, kind="ExternalInput").ap()
    Wreo = nc.dram_tensor("Wreo", [512, 512], dt.float32, kind="ExternalInput").ap()
    Wime = nc.dram_tensor("Wime", [512, 640], dt.float32, kind="ExternalInput").ap()
    Wimo = nc.dram_tensor("Wimo", [640, 512], dt.float32, kind="ExternalInput").ap()
    Cie = nc.dram_tensor("Cie", [F, TE], dt.float32, kind="ExternalInput").ap()
    Sie = nc.dram_tensor("Sie", [F, TO], dt.float32, kind="ExternalInput").ap()
    out2 = nc.dram_tensor("out2", [NB, C, T], dt.float32, kind="ExternalOutput").ap()

    v2 = nc.dram_tensor("v2", [NB * C, 2 * T], dt.float32).ap()  # internal

    with tile.TileContext(nc) as tc:
        from contextlib import ExitStack
        for b in range(NB):
            es_qk, es_p, es_r, es_agg = ExitStack(), ExitStack(), ExitStack(), ExitStack()
            qk_pool = es_qk.enter_context(tc.tile_pool(name=f"qk{b}", bufs=1, side="left"))
            qTee = qk_pool.tile([P, 5, C], dt.float32, tag="qTee")
            qTeo = qk_pool.tile([P, 4, C], dt.float32, tag="qTeo")
            qToo = qk_pool.tile([P, 4, C], dt.float32, tag="qToo")
            qToe = qk_pool.tile([P, 5, C], dt.float32, tag="qToe")
            kTee = qk_pool.tile([P, 5, C], dt.float32, tag="kTee")
            kTeo = qk_pool.tile([P, 4, C], dt.float32, tag="kTeo")
            kToo = qk_pool.tile([P, 4, C], dt.float32, tag="kToo")
            kToe = qk_pool.tile([P, 5, C], dt.float32, tag="kToe")

            # ---- A: radix split + projections ----
            # xee/xeo/xoo/xoe from x via paired sums around pivots T/2 and T/4
            with tc.tile_pool(name=f"a{b}", bufs=1) as ap_, \
                 tc.tile_pool(name=f"aps{b}", bufs=3, space="PSUM") as aps:
                for name, srcx, wsrc, dsts in (
                    ("q", query2, Wq, (None,)), ("k", key2, Wk, (None,))):
                    dee, deo, doo, doe = ((qTee, qTeo, qToo, qToe)
                                          if name == "q" else
                                          (kTee, kTeo, kToo, kToe))
                    x_sb = ap_.tile([P, CC, T], dt.float32, tag="x_sb")
                    nc.sync.dma_start(
                        x_sb[:], srcx[b].rearrange("(n p) t -> p n t", p=P))
                    w_sb = ap_.tile([P, CC, C], dt.float32, tag=f"w_{name}")
                    nc.sync.dma_start(
                        w_sb[:], wsrc.rearrange("(n p) d -> p n d", p=P))
                    xee = ap_.tile([P, CC, 640], dt.float32, tag="xee")
                    xeo = ap_.tile([P, CC, 512], dt.float32, tag="xeo")
                    xoo = ap_.tile([P, CC, 512], dt.float32, tag="xoo")
                    xoe = ap_.tile([P, CC, 640], dt.float32, tag="xoe")
                    ab = ap_.tile([P, 4, 511], dt.float32, tag="ab")
                    for cc in range(CC):
                        x = x_sb[:, cc, :]
                        nc.gpsimd.memset(xee[:, cc, 513:640], 0.0)
                        nc.gpsimd.memset(xoe[:, cc, 513:640], 0.0)
                        nc.gpsimd.memset(xoe[:, cc, 0:1], 0.0)
                        nc.gpsimd.memset(xoo[:, cc, 0:1], 0.0)
                        # a=x[1:512]+x[2047:1536:-1], b=x[1023:512:-1]+x[1025:1536]
                        # c,d likewise with minus
                        nc.vector.tensor_tensor(out=ab[:, 0, :], in0=x[:, 1:512],
                                                in1=x[:, T - 1:1536:-1], op=OP.add)
                        nc.vector.tensor_tensor(out=ab[:, 1, :], in0=x[:, 1023:512:-1],
                                                in1=x[:, 1025:1536], op=OP.add)
                        nc.vector.tensor_tensor(out=ab[:, 2, :], in0=x[:, 1:512],
                                                in1=x[:, T - 1:1536:-1], op=OP.subtract)
                        nc.vector.tensor_tensor(out=ab[:, 3, :], in0=x[:, 1023:512:-1],
                                                in1=x[:, 1025:1536], op=OP.subtract)
                        nc.vector.tensor_tensor(out=xee[:, cc, 1:512], in0=ab[:, 0, :],
                                                in1=ab[:, 1, :], op=OP.add)
                        nc.vector.tensor_tensor(out=xeo[:, cc, 1:512], in0=ab[:, 0, :],
                                                in1=ab[:, 1, :], op=OP.subtract)
                        nc.vector.tensor_tensor(out=xoo[:, cc, 1:512], in0=ab[:, 2, :],
                                                in1=ab[:, 3, :], op=OP.subtract)
                        nc.vector.tensor_tensor(out=xoe[:, cc, 1:512], in0=ab[:, 2, :],
                                                in1=ab[:, 3, :], op=OP.add)
                        # ends: xee[0]=x0+x1024, xeo[0]=x0-x1024,
                        #       xee[512]=x512+x1536, xoe[512]=x512-x1536
                        nc.vector.tensor_tensor(out=xee[:, cc, 0:1], in0=x[:, 0:1],
                                                in1=x[:, H:H + 1], op=OP.add)
                        nc.vector.tensor_tensor(out=xeo[:, cc, 0:1], in0=x[:, 0:1],
                                                in1=x[:, H:H + 1], op=OP.subtract)
                        nc.vector.tensor_tensor(out=xee[:, cc, 512:513], in0=x[:, 512:513],
                                                in1=x[:, 1536:1537], op=OP.add)
                        nc.vector.tensor_tensor(out=xoe[:, cc, 512:513], in0=x[:, 512:513],
                                                in1=x[:, 1536:1537], op=OP.subtract)
                    for st_, dst, nch in ((xee, dee, 5), (xeo, deo, 4),
                                          (xoo, doo, 4), (xoe, doe, 5)):
                        for i in range(nch):
                            ps = aps.tile([P, C], dt.float32, tag="proj_ps")
                            for cc in range(CC):
                                nc.tensor.matmul(
                                    ps[:], st_[:, cc, bass.ts(i, P)],
                                    w_sb[:, cc, :],
                                    start=(cc == 0), stop=(cc == CC - 1))
                            nc.scalar.activation(dst[:, i, :], ps[:], AF.Copy)

                # v projection: v[d,t], then duplicate into v2
                x_sb = ap_.tile([P, CC, T], dt.float32, tag="x_sb")
                nc.sync.dma_start(
                    x_sb[:], value2[b].rearrange("(n p) t -> p n t", p=P))
                w_sb = ap_.tile([P, CC, C], dt.float32, tag="w_v")
                nc.sync.dma_start(
                    w_sb[:], Wv.rearrange("(n p) d -> p n d", p=P))
                v2r = v2.rearrange("(n p) w -> n p w", p=P)
                for dc in range(CC):
                    v_sb = ap_.tile([P, T], dt.float32, tag="v_sb")
                    for tb in range(4):
                        ps = aps.tile([P, T // 4], dt.float32, tag="v_ps")
                        for cc in range(CC):
                            nc.tensor.matmul(
                                ps[:], w_sb[:, cc, bass.ts(dc, P)],
                                x_sb[:, cc, bass.ts(tb, T // 4)],
                                start=(cc == 0), stop=(cc == CC - 1))
                        nc.scalar.activation(
                            v_sb[:, bass.ts(tb, T // 4)], ps[:], AF.Copy)
                    nc.sync.dma_start(v2r[b * CC + dc, :, 0:T], v_sb[:])
                    nc.sync.dma_start(v2r[b * CC + dc, :, T:2 * T], v_sb[:])

            # ---- B: forward DFT + pointwise ----
            p_pool = es_p.enter_context(tc.tile_pool(name=f"p{b}", bufs=1, side="right"))
            pre = p_pool.tile([P, FC, C], dt.float32, tag="pre")
            pim = p_pool.tile([P, FC, C], dt.float32, tag="pim")
            with tc.tile_pool(name=f"bmat{b}", bufs=3) as bmat, \
                 tc.tile_pool(name=f"bps{b}", bufs=2, space="PSUM") as bps, \
                 tc.tile_pool(name=f"btmp{b}", bufs=2) as btmp:
                wree_r = Wree.rearrange("(n p) f -> p n f", p=P)   # [128,5,640]
                wreo_r = Wreo.rearrange("(n p) f -> p n f", p=P)   # [128,4,512]
                wime_r = Wime.rearrange("(n p) f -> p n f", p=P)   # [128,4,640]
                wimo_r = Wimo.rearrange("(n p) f -> p n f", p=P)   # [128,5,512]
                for fc in range(FC):
                    even = fc < 5
                    fl = fc if even else fc - 5
                    ncos, nsin = (5, 4) if even else (4, 5)
                    cm = bmat.tile([P, 5, P], dt.float32, tag="cm")
                    nc.sync.dma_start(
                        cm[:, 0:ncos, :],
                        (wree_r if even else wreo_r)[:, :, bass.ts(fl, P)])
                    sm = bmat.tile([P, 5, P], dt.float32, tag="sm")
                    nc.sync.dma_start(
                        sm[:, 0:nsin, :],
                        (wime_r if even else wimo_r)[:, :, bass.ts(fl, P)])
                    qcos = qTee if even else qTeo
                    qsin = qToo if even else qToe
                    kcos = kTee if even else kTeo
                    ksin = kToo if even else kToe
                    acc = {}
                    for nm, mat, sig, nchunk in (
                            ("aq", cm, qcos, ncos), ("bq", sm, qsin, nsin),
                            ("ak", cm, kcos, ncos), ("bk", sm, ksin, nsin)):
                        ps = bps.tile([P, C], dt.float32, tag=nm, name=f"ps_{nm}")
                        for i in range(nchunk):
                            nc.tensor.matmul(
                                ps[:], mat[:, i, :], sig[:, i, :],
                                start=(i == 0), stop=(i == nchunk - 1))
                        acc[nm] = ps
                    # DVE reads at most one PSUM operand: stage aq/bq in SBUF
                    aqs = btmp.tile([P, C], dt.float32, tag="aqs")
                    nc.scalar.activation(aqs[:], acc["aq"][:], AF.Copy)
                    bqs = btmp.tile([P, C], dt.float32, tag="bqs")
                    nc.scalar.activation(bqs[:], acc["bq"][:], AF.Copy)
                    tmp = btmp.tile([P, C], dt.float32, tag="tmp")
                    nc.vector.tensor_tensor(
                        out=pre[:, fc, :], in0=aqs[:],
                        in1=acc["ak"][:], op=OP.mult)
                    nc.vector.tensor_tensor(
                        out=tmp[:], in0=bqs[:], in1=acc["bk"][:],
                        op=OP.mult)
                    nc.vector.tensor_tensor(
                        out=pre[:, fc, :], in0=pre[:, fc, :],
                        in1=tmp[:], op=OP.add)
                    nc.vector.tensor_tensor(
                        out=pim[:, fc, :], in0=bqs[:],
                        in1=acc["ak"][:], op=OP.mult)
                    tmp2 = btmp.tile([P, C], dt.float32, tag="tmp2")
                    nc.vector.tensor_tensor(
                        out=tmp2[:], in0=aqs[:], in1=acc["bk"][:],
                        op=OP.mult)
                    nc.vector.tensor_tensor(
                        out=pim[:, fc, :], in0=pim[:, fc, :],
                        in1=tmp2[:], op=OP.subtract)
            es_qk.close()  # qT/kT no longer needed

            # ---- C: inverse DFT -> r[c, t] ----
            # rc[t]=sum_f Pre*cie (t=0..1024), rs[t]=sum_f Pim*sie (t=0..1023)
            # r[0:1024]=rc+rs, r[1024]=rc[1024], r[2048-j]=rc[j]-rs[j] j=1..1023
            r_pool = es_r.enter_context(tc.tile_pool(name=f"r{b}", bufs=1, side="left"))
            r_sb = [r_pool.tile([P, T], dt.float32, tag=f"r{cc}", name=f"r_sb{cc}")
                    for cc in range(CC)]
            with tc.tile_pool(name=f"cmat{b}", bufs=1) as cmat, \
                 tc.tile_pool(name=f"ctmp{b}", bufs=2) as ctmp, \
                 tc.tile_pool(name=f"cps{b}", bufs=1, space="PSUM") as cps:
                # f-parity split: pre/pim chunks 0:5 = even f, 5:9 = odd f.
                # Mirror identities make only t<=512 columns needed:
                #   rc[t]=rcE[t]+rcO[t], rc[1024-t]=rcE[t]-rcO[t]
                #   rs[t]=rsE[t]+rsO[t], rs[1024-t]=rsO[t]-rsE[t]
                cie_sb = cmat.tile([P, FC, 640], dt.float32, tag="cie_sb")
                nc.sync.dma_start(
                    cie_sb[:], Cie.rearrange("(n p) t -> p n t", p=P)[:, :, 0:640])
                sie_sb = cmat.tile([P, FC, 640], dt.float32, tag="sie_sb")
                nc.sync.dma_start(
                    sie_sb[:], Sie.rearrange("(n p) t -> p n t", p=P)[:, :, 0:640])
                HB = H // 2  # 512
                for cc in range(CC):
                    rcE = cps.tile([P, HB], dt.float32, tag="rcE", name="ps_rcE")
                    rcE2 = cps.tile([P, P], dt.float32, tag="rcE2", name="ps_rcE2")
                    rcO = cps.tile([P, HB], dt.float32, tag="rcO", name="ps_rcO")
                    rsE = cps.tile([P, HB], dt.float32, tag="rsE", name="ps_rsE")
                    rsO = cps.tile([P, HB], dt.float32, tag="rsO", name="ps_rsO")
                    rsO2 = cps.tile([P, P], dt.float32, tag="rsO2", name="ps_rsO2")
                    for fc in range(5):
                        st, sp = (fc == 0), (fc == 4)
                        pre_l = pre[:, fc, bass.ts(cc, P)]
                        pim_l = pim[:, fc, bass.ts(cc, P)]
                        nc.tensor.matmul(rcE[:], pre_l, cie_sb[:, fc, 0:HB],
                                         start=st, stop=sp)
                        nc.tensor.matmul(rcE2[:], pre_l, cie_sb[:, fc, HB:HB + P],
                                         start=st, stop=sp)
                        nc.tensor.matmul(rsE[:], pim_l, sie_sb[:, fc, 0:HB],
                                         start=st, stop=sp)
                    for fc in range(5, FC):
                        st, sp = (fc == 5), (fc == FC - 1)
                        pre_l = pre[:, fc, bass.ts(cc, P)]
                        pim_l = pim[:, fc, bass.ts(cc, P)]
                        nc.tensor.matmul(rcO[:], pre_l, cie_sb[:, fc, 0:HB],
                                         start=st, stop=sp)
                        nc.tensor.matmul(rsO[:], pim_l, sie_sb[:, fc, 0:HB],
                                         start=st, stop=sp)
                        nc.tensor.matmul(rsO2[:], pim_l, sie_sb[:, fc, HB:HB + P],
                                         start=st, stop=sp)
                    rcO_sb = ctmp.tile([P, HB], dt.float32, tag="rcO_sb")
                    nc.scalar.activation(rcO_sb[:], rcO[:], AF.Copy)
                    rsE_sb = ctmp.tile([P, HB], dt.float32, tag="rsE_sb")
                    nc.scalar.activation(rsE_sb[:], rsE[:], AF.Copy)
                    rsO_sb = ctmp.tile([P, HB + 1], dt.float32, tag="rsO_sb")
                    nc.scalar.activation(rsO_sb[:, 0:HB], rsO[:], AF.Copy)
                    nc.scalar.activation(rsO_sb[:, HB:HB + 1], rsO2[:, 0:1], AF.Copy)
                    s1 = ctmp.tile([P, HB], dt.float32, tag="s1")
                    nc.vector.tensor_tensor(out=s1[:], in0=rcE[:], in1=rcO_sb[:],
                                            op=OP.add)
                    s2 = ctmp.tile([P, HB], dt.float32, tag="s2")
                    nc.vector.tensor_tensor(out=s2[:], in0=rcE[:], in1=rcO_sb[:],
                                            op=OP.subtract)
                    w1 = ctmp.tile([P, HB], dt.float32, tag="w1")
                    nc.vector.tensor_tensor(out=w1[:], in0=rsE_sb[:],
                                            in1=rsO_sb[:, 0:HB], op=OP.add)
                    w2 = ctmp.tile([P, HB], dt.float32, tag="w2")
                    nc.vector.tensor_tensor(out=w2[:], in0=rsO_sb[:, 0:HB],
                                            in1=rsE_sb[:], op=OP.subtract)
                    rt = r_sb[cc]
                    nc.vector.tensor_tensor(out=rt[:, 0:HB], in0=s1[:], in1=w1[:],
                                            op=OP.add)
                    nc.vector.tensor_tensor(out=rt[:, 1023:HB:-1], in0=s2[:, 1:HB],
                                            in1=w2[:, 1:HB], op=OP.add)
                    nc.vector.tensor_tensor(out=rt[:, 1025:1536], in0=s2[:, 1:HB],
                                            in1=w2[:, 1:HB], op=OP.subtract)
                    nc.vector.tensor_tensor(out=rt[:, T - 1:1536:-1], in0=s1[:, 1:HB],
                                            in1=w1[:, 1:HB], op=OP.subtract)
                    nc.vector.tensor_tensor(out=rt[:, HB:HB + 1], in0=rcE2[:, 0:1],
                                            in1=rsO_sb[:, HB:HB + 1], op=OP.add)
                    nc.vector.tensor_tensor(out=rt[:, H:H + 1], in0=rcE[:, 0:1],
                                            in1=rcO_sb[:, 0:1], op=OP.subtract)
                    nc.vector.tensor_tensor(out=rt[:, 1536:1537], in0=rcE2[:, 0:1],
                                            in1=rsO_sb[:, HB:HB + 1], op=OP.subtract)
            es_p.close()  # pre/pim no longer needed

            # ---- D: top-k, softmax weights, gather ----
            agg_pool = es_agg.enter_context(tc.tile_pool(name=f"agg{b}", bufs=1, side="right"))
            agg = [agg_pool.tile([P, T], dt.float32, tag=f"agg{j}", name=f"agg_sb{j}")
                   for j in range(NE)]
            with tc.tile_pool(name=f"d{b}", bufs=2) as dp:
                for cc in range(CC):
                    vals = dp.tile([P, 8], dt.float32, tag="vals")
                    idx = dp.tile([P, 8], dt.uint32, tag="idx")
                    nc.vector.max(vals[:], r_sb[cc][:])
                    nc.vector.max_index(idx[:], vals[:], r_sb[cc][:])
                    negm = dp.tile([P, 1], dt.float32, tag="negm")
                    nc.scalar.activation(negm[:], vals[:, 0:1],
                                         AF.Copy, bias=0.0, scale=-1.0)
                    esc = dp.tile([P, T], dt.float32, tag="esc")
                    s_col = dp.tile([P, 1], dt.float32, tag="s_col")
                    nc.scalar.activation(
                        esc[:], r_sb[cc][:], AF.Exp,
                        bias=negm[:, 0:1], scale=1.0,
                        accum_out=s_col[:, 0:1])
                    rs = dp.tile([P, 1], dt.float32, tag="rs")
                    nc.vector.reciprocal(rs[:], s_col[:])
                    ew = dp.tile([P, K], dt.float32, tag="ew")
                    nc.scalar.activation(ew[:], vals[:, 0:K],
                                         AF.Exp, bias=negm[:, 0:1],
                                         scale=1.0)
                    w3 = dp.tile([P, K], dt.float32, tag="w3")
                    nc.vector.tensor_scalar_mul(w3[:], ew[:], rs[:, 0:1])

                    # gather offsets: (b*512+cc*128+p)*4096 + 2048 - lag
                    iot = dp.tile([P, 1], dt.int32, tag="iot")
                    nc.gpsimd.iota(
                        iot[:], pattern=[[0, 1]],
                        base=(b * C + cc * P) * (2 * T) + T,
                        channel_multiplier=2 * T)
                    iot_f = dp.tile([P, 1], dt.float32, tag="iot_f")
                    nc.vector.tensor_copy(iot_f[:], iot[:])
                    idx_f = dp.tile([P, K], dt.float32, tag="idx_f")
                    nc.vector.tensor_copy(idx_f[:], idx[:, 0:K])
                    gof = dp.tile([P, K], dt.float32, tag="gof")
                    nc.scalar.activation(gof[:], idx_f[:],
                                         AF.Copy, bias=0.0, scale=-1.0)
                    nc.vector.tensor_scalar_add(gof[:], gof[:],
                                                iot_f[:, 0:1])
                    gou = dp.tile([P, K], dt.uint32, tag="gou")
                    nc.vector.tensor_copy(gou[:], gof[:])

                    for k in range(K):
                        a_t = agg[k * CC + cc]
                        for hh in range(2):
                            nc.gpsimd.indirect_dma_start(
                                out=a_t[:, bass.ts(hh, T // 2)], out_offset=None,
                                in_=v2[:, :],
                                in_offset=bass.IndirectOffsetOnAxis(
                                    ap=gou[:, k:k + 1], axis=1),
                                element_offset=hh * (T // 2))
                        nc.vector.tensor_scalar_mul(
                            a_t[:], a_t[:], w3[:, k:k + 1])
            es_r.close()  # r tiles no longer needed

            # ---- E: final projection ----
            with tc.tile_pool(name=f"e{b}", bufs=1) as ep, \
                 tc.tile_pool(name=f"eps{b}", bufs=8, space="PSUM") as eps:
                wf_sb = ep.tile([P, NE, C], dt.float32, tag="wf_sb")
                nc.sync.dma_start(
                    wf_sb[:], Wf.rearrange("(n p) d -> p n d", p=P))
                for dc in range(CC):
                    for tb in range(4):
                        ps = eps.tile([P, T // 4], dt.float32,
                                      tag="out_ps")
                        for j in range(NE):
                            nc.tensor.matmul(
                                ps[:], wf_sb[:, j, bass.ts(dc, P)],
                                agg[j][:, bass.ts(tb, T // 4)],
                                start=(j == 0), stop=(j == NE - 1))
                        o_sb = ep.tile([P, T // 4], dt.float32,
                                       tag="o_sb")
                        nc.scalar.activation(o_sb[:], ps[:], AF.Copy)
                        nc.sync.dma_start(
                            out2[b, bass.ts(dc, P),
                                 bass.ts(tb, T // 4)], o_sb[:])
            es_agg.close()

    nc.compile()
    return nc


def _get_nc():
    if "nc" not in _CACHE:
        _CACHE["nc"] = _build()
    return _CACHE["nc"]


def kernel(query, key, value, Wq, bq, Wk, bk, Wv, bv, Wf, bf):
    query = np.ascontiguousarray(np.asarray(query, dtype=np.float32))
    key = np.ascontiguousarray(np.asarray(key, dtype=np.float32))
    value = np.ascontiguousarray(np.asarray(value, dtype=np.float32))
    for bias in (bq, bk, bv, bf):
        assert np.all(np.asarray(bias) == 0.0), "nonzero biases unsupported"

    if "mats" not in _CACHE:
        _CACHE["mats"] = _dft_matrices()
    wree, wreo, wime, wimo, cie, sie = _CACHE["mats"]

    shared = {
        "Wq": np.ascontiguousarray(np.asarray(Wq, np.float32)),
        "Wk": np.ascontiguousarray(np.asarray(Wk, np.float32)),
        "Wv": np.ascontiguousarray(np.asarray(Wv, np.float32)),
        "Wf": np.ascontiguousarray(np.asarray(Wf, np.float32)),
        "Wree": wree, "Wreo": wreo, "Wime": wime, "Wimo": wimo,
        "Cie": cie, "Sie": sie,
    }
    in_maps = []
    for c in range(NCORES):
        sl = slice(c * NB, (c + 1) * NB)
        in_maps.append({
            "query2": query[sl], "key2": key[sl], "value2": value[sl], **shared})

    from concourse.bass_utils import run_bass_kernel_spmd
    nc = _get_nc()
    res = run_bass_kernel_spmd(nc, in_maps, core_ids=list(range(NCORES)))
    _CACHE["last_results"] = res
    out = np.concatenate([res.results[c]["out2"] for c in range(NCORES)], axis=0)
    return out.astype(np.float32)



# revision 3
# speedup vs baseline: 1.3389x; 1.3389x over previous
"""AutoCorrelation (Autoformer-style) Bass kernel for Trainium2, 8 NeuronCores.

Full inputs in, full outputs out. Data-parallel over batch: B=16 -> 2 batches
per core. Per core, per batch:

  A. Channel projections on the PE:
       qT[t,d] = sum_c query[c,t] Wq[c,d]   (t on partitions -> feeds DFT)
       kT[t,d] likewise
       v[d,t]  = sum_c Wv[c,d] value[c,t]   (d on partitions -> row gather)
     v is written twice side-by-side into DRAM table v2[b*512+d, 4096] for
     circular-shift gathers.
  B. Forward real DFT via matmuls with cos/-sin matrices (F padded to 1152),
     fused pointwise P = FQ * conj(FK) on the DVE.
  C. Inverse DFT via matmuls: r[c,t] = sum_f Pre*ci + Pim*sn, where ci/sn
     carry the irfft weights (1,2,...,2,1)/T^2 and sn = -w*sin/T^2.
  D. Per 128-channel tile: top-8 values+indices (InstMax/InstMaxIndex),
     softmax weights of the top-3 straight from the top values
     (w_k = exp(v_k - v_0) / sum_t exp(r - v_0)), circular-shift rows of v
     via indirect-DMA gather with flat element offsets into agg[k*C+c, t],
     scaled by w_k.
  E. out[d,t] = sum_e Wf[e,d] agg[e,t] via 12-chunk PE accumulation.

All matmuls run in fp32: the top-3 lag selection needs auto_corr to match the
reference's fp32 FFT to ~1e-7 absolute; fp32 PE matmuls measure ~2.4e-7
relative which lands flips at the same rate as the reference's own fp32 error.
bf16/tf32-grade products would scramble the top-3 ordering entirely.

Biases are all zero in this problem's setup_inputs(); asserted host-side.
"""
import numpy as np

import concourse.bass as bass
import concourse.tile as tile
from concourse import bacc, mybir

dt = mybir.dt
AF = mybir.ActivationFunctionType
OP = mybir.AluOpType

P = 128
B, C, T, K = 16, 512, 2048, 3
NB = 2                    # batches per core
NCORES = 8
F = 1152                  # rfft bins 1025, padded to 9*128
TC = T // P               # 16 t-chunks
CC = C // P               # 4 c-chunks
FC = F // P               # 9 f-chunks
NE = K * C // P           # 12 e-chunks of Wf / agg
TE = 1152                 # even-part length 1025, padded to 9*128
TEC = TE // P             # 9
TO = 1024                 # odd-part length (t=1..1023 live)
TOC = TO // P             # 8
H = T // 2                # 1024

_CACHE = {}


def _dft_matrices():
    """Radix-split DFT matrices (fp64 -> fp32).

    Level-1 even/odd in t (qe/qo), then level-2 split by f parity:
      FQre over even f contracts xee (t=0..512), odd f contracts xeo (t=0..511)
      FQim over even f contracts xoo (t=1..511), odd f contracts xoe (t=1..512)
    Frequency storage is parity-permuted: chunks [0:5]=even f (2g, g<=512),
    chunks [5:9]=odd f (2g+1). Inverse matrices have rows permuted to match.
    """
    t640 = np.arange(640.0)[:, None]
    t512 = np.arange(512.0)[:, None]
    ge = np.arange(640.0)[None, :]
    go = np.arange(512.0)[None, :]
    wree = np.where((t640 <= 512) & (ge <= 512),
                    np.cos(2 * np.pi * t640 * (2 * ge) / T), 0.0).astype(np.float32)
    wreo = np.cos(2 * np.pi * t512 * (2 * go + 1) / T).astype(np.float32)
    wime = np.where(ge <= 512,
                    -np.sin(2 * np.pi * t512 * (2 * ge) / T), 0.0).astype(np.float32)
    wimo = np.where(t640 <= 512,
                    -np.sin(2 * np.pi * t640 * (2 * go + 1) / T), 0.0).astype(np.float32)

    f64 = np.arange(F, dtype=np.float64)[None, :]
    livef = f64 <= H
    w = np.where((f64 == 0) | (f64 == H), 1.0, 2.0) * livef / (T * T)
    fc_ = f64.T
    tt = np.arange(TE, dtype=np.float64)[None, :]
    cie = np.where((fc_ <= H) & (tt <= H),
                   np.cos(2 * np.pi * fc_ * tt / T) * w.T, 0.0)
    tt2 = np.arange(TO, dtype=np.float64)[None, :]
    sie = np.where(fc_ <= H,
                   -np.sin(2 * np.pi * fc_ * tt2 / T) * w.T, 0.0)

    def permrows(m):
        out = np.zeros_like(m)
        out[0:513] = m[0:1025:2]
        out[640:1152] = m[1:1024:2]
        return out

    return (wree, wreo, wime, wimo,
            permrows(cie).astype(np.float32), permrows(sie).astype(np.float32))


def _build():
    nc = bacc.Bacc("TRN2", target_bir_lowering=False, debug=False,
                   num_devices=NCORES)

    query2 = nc.dram_tensor("query2", [NB, C, T], dt.float32, kind="ExternalInput").ap()
    key2 = nc.dram_tensor("key2", [NB, C, T], dt.float32, kind="ExternalInput").ap()
    value2 = nc.dram_tensor("value2", [NB, C, T], dt.float32r, kind="ExternalInput").ap()
    Wq = nc.dram_tensor("Wq", [C, C], dt.float32, kind="ExternalInput").ap()
    Wk = nc.dram_tensor("Wk", [C, C], dt.float32, kind="ExternalInput").ap()
    Wv = nc.dram_tensor("Wv", [C, C], dt.float32r, kind="ExternalInput").ap()
    Wf = nc.dram_tensor("Wf", [K * C, C], dt.float32r, kind="ExternalInput").ap()
    Wree = nc.dram_tensor("Wree", [640, 640], dt.float32, kind="ExternalInput").ap()
    Wreo = nc.dram_tensor("Wreo", [512, 512], dt.float32, kind="ExternalInput").ap()
    Wime = nc.dram_tensor("Wime", [512, 640], dt.float32, kind="ExternalInput").ap()
    Wimo = nc.dram_tensor("Wimo", [640, 512], dt.float32, kind="ExternalInput").ap()
    Cie = nc.dram_tensor("Cie", [F, TE], dt.float32, kind="ExternalInput").ap()
    Sie = nc.dram_tensor("Sie", [F, TO], dt.float32, kind="ExternalInput").ap()
    out2 = nc.dram_tensor("out2", [NB, C, T], dt.float32, kind="ExternalOutput").ap()

    v2 = nc.dram_tensor("v2", [NB * C, 2 * T], dt.float32r).ap()  # internal

    with tile.TileContext(nc) as tc:
        from contextlib import ExitStack
        for b in range(NB):
            es_qk, es_p, es_r, es_agg = ExitStack(), ExitStack(), ExitStack(), ExitStack()
            qk_pool = es_qk.enter_context(tc.tile_pool(name=f"qk{b}", bufs=1, side="left"))
            qTee = qk_pool.tile([P, 5, C], dt.float32, tag="qTee")
            qTeo = qk_pool.tile([P, 4, C], dt.float32, tag="qTeo")
            qToo = qk_pool.tile([P, 4, C], dt.float32, tag="qToo")
            qToe = qk_pool.tile([P, 5, C], dt.float32, tag="qToe")
            kTee = qk_pool.tile([P, 5, C], dt.float32, tag="kTee")
            kTeo = qk_pool.tile([P, 4, C], dt.float32, tag="kTeo")
            kToo = qk_pool.tile([P, 4, C], dt.float32, tag="kToo")
            kToe = qk_pool.tile([P, 5, C], dt.float32, tag="kToe")

            # ---- A: radix split + projections ----
            # xee/xeo/xoo/xoe from x via paired sums around pivots T/2 and T/4
            with tc.tile_pool(name=f"a{b}", bufs=1) as ap_, \
                 tc.tile_pool(name=f"aps{b}", bufs=3, space="PSUM") as aps:
                for name, srcx, wsrc, dsts in (
                    ("q", query2, Wq, (None,)), ("k", key2, Wk, (None,))):
                    dee, deo, doo, doe = ((qTee, qTeo, qToo, qToe)
                                          if name == "q" else
                                          (kTee, kTeo, kToo, kToe))
                    x_sb = ap_.tile([P, CC, T], dt.float32, tag="x_sb")
                    nc.sync.dma_start(
                        x_sb[:], srcx[b].rearrange("(n p) t -> p n t", p=P))
                    w_sb = ap_.tile([P, CC, C], dt.float32, tag=f"w_{name}")
                    nc.sync.dma_start(
                        w_sb[:], wsrc.rearrange("(n p) d -> p n d", p=P))
                    xee = ap_.tile([P, CC, 640], dt.float32, tag="xee")
                    xeo = ap_.tile([P, CC, 512], dt.float32, tag="xeo")
                    xoo = ap_.tile([P, CC, 512], dt.float32, tag="xoo")
                    xoe = ap_.tile([P, CC, 640], dt.float32, tag="xoe")
                    ab = ap_.tile([P, 4, 511], dt.float32, tag="ab")
                    for cc in range(CC):
                        x = x_sb[:, cc, :]
                        nc.gpsimd.memset(xee[:, cc, 513:640], 0.0)
                        nc.gpsimd.memset(xoe[:, cc, 513:640], 0.0)
                        nc.gpsimd.memset(xoe[:, cc, 0:1], 0.0)
                        nc.gpsimd.memset(xoo[:, cc, 0:1], 0.0)
                        # a=x[1:512]+x[2047:1536:-1], b=x[1023:512:-1]+x[1025:1536]
                        # c,d likewise with minus
                        nc.vector.tensor_tensor(out=ab[:, 0, :], in0=x[:, 1:512],
                                                in1=x[:, T - 1:1536:-1], op=OP.add)
                        nc.vector.tensor_tensor(out=ab[:, 1, :], in0=x[:, 1023:512:-1],
                                                in1=x[:, 1025:1536], op=OP.add)
                        nc.vector.tensor_tensor(out=ab[:, 2, :], in0=x[:, 1:512],
                                                in1=x[:, T - 1:1536:-1], op=OP.subtract)
                        nc.vector.tensor_tensor(out=ab[:, 3, :], in0=x[:, 1023:512:-1],
                                                in1=x[:, 1025:1536], op=OP.subtract)
                        nc.vector.tensor_tensor(out=xee[:, cc, 1:512], in0=ab[:, 0, :],
                                                in1=ab[:, 1, :], op=OP.add)
                        nc.vector.tensor_tensor(out=xeo[:, cc, 1:512], in0=ab[:, 0, :],
                                                in1=ab[:, 1, :], op=OP.subtract)
                        nc.vector.tensor_tensor(out=xoo[:, cc, 1:512], in0=ab[:, 2, :],
                                                in1=ab[:, 3, :], op=OP.subtract)
                        nc.vector.tensor_tensor(out=xoe[:, cc, 1:512], in0=ab[:, 2, :],
                                                in1=ab[:, 3, :], op=OP.add)
                        # ends: xee[0]=x0+x1024, xeo[0]=x0-x1024,
                        #       xee[512]=x512+x1536, xoe[512]=x512-x1536
                        nc.vector.tensor_tensor(out=xee[:, cc, 0:1], in0=x[:, 0:1],
                                                in1=x[:, H:H + 1], op=OP.add)
                        nc.vector.tensor_tensor(out=xeo[:, cc, 0:1], in0=x[:, 0:1],
                                                in1=x[:, H:H + 1], op=OP.subtract)
                        nc.vector.tensor_tensor(out=xee[:, cc, 512:513], in0=x[:, 512:513],
                                                in1=x[:, 1536:1537], op=OP.add)
                        nc.vector.tensor_tensor(out=xoe[:, cc, 512:513], in0=x[:, 512:513],
                                                in1=x[:, 1536:1537], op=OP.subtract)
                    for st_, dst, nch in ((xee, dee, 5), (xeo, deo, 4),
                                          (xoo, doo, 4), (xoe, doe, 5)):
                        for i in range(nch):
                            ps = aps.tile([P, C], dt.float32, tag="proj_ps")
                            for cc in range(CC):
                                nc.tensor.matmul(
                                    ps[:], st_[:, cc, bass.ts(i, P)],
                                    w_sb[:, cc, :],
                                    start=(cc == 0), stop=(cc == CC - 1))
                            nc.scalar.activation(dst[:, i, :], ps[:], AF.Copy)

                # v projection: v[d,t], then duplicate into v2
                x_sb = ap_.tile([P, CC, T], dt.float32r, tag="x_sb")
                nc.sync.dma_start(
                    x_sb[:], value2[b].rearrange("(n p) t -> p n t", p=P))
                w_sb = ap_.tile([P, CC, C], dt.float32r, tag="w_v")
                nc.sync.dma_start(
                    w_sb[:], Wv.rearrange("(n p) d -> p n d", p=P))
                v2r = v2.rearrange("(n p) w -> n p w", p=P)
                for dc in range(CC):
                    v_sb = ap_.tile([P, T], dt.float32r, tag="v_sb")
                    for tb in range(4):
                        ps = aps.tile([P, T // 4], dt.float32, tag="v_ps")
                        for cc in range(CC):
                            nc.tensor.matmul(
                                ps[:], w_sb[:, cc, bass.ts(dc, P)],
                                x_sb[:, cc, bass.ts(tb, T // 4)],
                                start=(cc == 0), stop=(cc == CC - 1))
                        nc.scalar.activation(
                            v_sb[:, bass.ts(tb, T // 4)], ps[:], AF.Copy)
                    nc.sync.dma_start(v2r[b * CC + dc, :, 0:T], v_sb[:])
                    nc.sync.dma_start(v2r[b * CC + dc, :, T:2 * T], v_sb[:])

            # ---- B: forward DFT + pointwise ----
            p_pool = es_p.enter_context(tc.tile_pool(name=f"p{b}", bufs=1, side="right"))
            pre = p_pool.tile([P, FC, C], dt.float32, tag="pre")
            pim = p_pool.tile([P, FC, C], dt.float32, tag="pim")
            with tc.tile_pool(name=f"bmat{b}", bufs=3) as bmat, \
                 tc.tile_pool(name=f"bps{b}", bufs=2, space="PSUM") as bps, \
                 tc.tile_pool(name=f"btmp{b}", bufs=2) as btmp:
                wree_r = Wree.rearrange("(n p) f -> p n f", p=P)   # [128,5,640]
                wreo_r = Wreo.rearrange("(n p) f -> p n f", p=P)   # [128,4,512]
                wime_r = Wime.rearrange("(n p) f -> p n f", p=P)   # [128,4,640]
                wimo_r = Wimo.rearrange("(n p) f -> p n f", p=P)   # [128,5,512]
                for fc in range(FC):
                    even = fc < 5
                    fl = fc if even else fc - 5
                    ncos, nsin = (5, 4) if even else (4, 5)
                    cm = bmat.tile([P, 5, P], dt.float32, tag="cm")
                    nc.sync.dma_start(
                        cm[:, 0:ncos, :],
                        (wree_r if even else wreo_r)[:, :, bass.ts(fl, P)])
                    sm = bmat.tile([P, 5, P], dt.float32, tag="sm")
                    nc.sync.dma_start(
                        sm[:, 0:nsin, :],
                        (wime_r if even else wimo_r)[:, :, bass.ts(fl, P)])
                    qcos = qTee if even else qTeo
                    qsin = qToo if even else qToe
                    kcos = kTee if even else kTeo
                    ksin = kToo if even else kToe
                    acc = {}
                    for nm, mat, sig, nchunk in (
                            ("aq", cm, qcos, ncos), ("bq", sm, qsin, nsin),
                            ("ak", cm, kcos, ncos), ("bk", sm, ksin, nsin)):
                        ps = bps.tile([P, C], dt.float32, tag=nm, name=f"ps_{nm}")
                        for i in range(nchunk):
                            nc.tensor.matmul(
                                ps[:], mat[:, i, :], sig[:, i, :],
                                start=(i == 0), stop=(i == nchunk - 1))
                        acc[nm] = ps
                    # DVE reads at most one PSUM operand: stage aq/bq in SBUF
                    aqs = btmp.tile([P, C], dt.float32, tag="aqs")
                    nc.scalar.activation(aqs[:], acc["aq"][:], AF.Copy)
                    bqs = btmp.tile([P, C], dt.float32, tag="bqs")
                    nc.scalar.activation(bqs[:], acc["bq"][:], AF.Copy)
                    tmp = btmp.tile([P, C], dt.float32, tag="tmp")
                    nc.vector.tensor_tensor(
                        out=pre[:, fc, :], in0=aqs[:],
                        in1=acc["ak"][:], op=OP.mult)
                    nc.vector.tensor_tensor(
                        out=tmp[:], in0=bqs[:], in1=acc["bk"][:],
                        op=OP.mult)
                    nc.vector.tensor_tensor(
                        out=pre[:, fc, :], in0=pre[:, fc, :],
                        in1=tmp[:], op=OP.add)
                    nc.vector.tensor_tensor(
                        out=pim[:, fc, :], in0=bqs[:],
                        in1=acc["ak"][:], op=OP.mult)
                    tmp2 = btmp.tile([P, C], dt.float32, tag="tmp2")
                    nc.vector.tensor_tensor(
                        out=tmp2[:], in0=aqs[:], in1=acc["bk"][:],
                        op=OP.mult)
                    nc.vector.tensor_tensor(
                        out=pim[:, fc, :], in0=pim[:, fc, :],
                        in1=tmp2[:], op=OP.subtract)
            es_qk.close()  # qT/kT no longer needed

            # ---- C: inverse DFT -> r[c, t] ----
            # rc[t]=sum_f Pre*cie (t=0..1024), rs[t]=sum_f Pim*sie (t=0..1023)
            # r[0:1024]=rc+rs, r[1024]=rc[1024], r[2048-j]=rc[j]-rs[j] j=1..1023
            r_pool = es_r.enter_context(tc.tile_pool(name=f"r{b}", bufs=1, side="left"))
            r_sb = [r_pool.tile([P, T], dt.float32, tag=f"r{cc}", name=f"r_sb{cc}")
                    for cc in range(CC)]
            with tc.tile_pool(name=f"cmat{b}", bufs=1) as cmat, \
                 tc.tile_pool(name=f"ctmp{b}", bufs=2) as ctmp, \
                 tc.tile_pool(name=f"cps{b}", bufs=1, space="PSUM") as cps:
                # f-parity split: pre/pim chunks 0:5 = even f, 5:9 = odd f.
                # Mirror identities make only t<=512 columns needed:
                #   rc[t]=rcE[t]+rcO[t], rc[1024-t]=rcE[t]-rcO[t]
                #   rs[t]=rsE[t]+rsO[t], rs[1024-t]=rsO[t]-rsE[t]
                cie_sb = cmat.tile([P, FC, 640], dt.float32, tag="cie_sb")
                nc.sync.dma_start(
                    cie_sb[:], Cie.rearrange("(n p) t -> p n t", p=P)[:, :, 0:640])
                sie_sb = cmat.tile([P, FC, 640], dt.float32, tag="sie_sb")
                nc.sync.dma_start(
                    sie_sb[:], Sie.rearrange("(n p) t -> p n t", p=P)[:, :, 0:640])
                HB = H // 2  # 512
                for cc in range(CC):
                    rcE = cps.tile([P, HB], dt.float32, tag="rcE", name="ps_rcE")
                    rcE2 = cps.tile([P, P], dt.float32, tag="rcE2", name="ps_rcE2")
                    rcO = cps.tile([P, HB], dt.float32, tag="rcO", name="ps_rcO")
                    rsE = cps.tile([P, HB], dt.float32, tag="rsE", name="ps_rsE")
                    rsO = cps.tile([P, HB], dt.float32, tag="rsO", name="ps_rsO")
                    rsO2 = cps.tile([P, P], dt.float32, tag="rsO2", name="ps_rsO2")
                    for fc in range(5):
                        st, sp = (fc == 0), (fc == 4)
                        pre_l = pre[:, fc, bass.ts(cc, P)]
                        pim_l = pim[:, fc, bass.ts(cc, P)]
                        nc.tensor.matmul(rcE[:], pre_l, cie_sb[:, fc, 0:HB],
                                         start=st, stop=sp)
                        nc.tensor.matmul(rcE2[:], pre_l, cie_sb[:, fc, HB:HB + P],
                                         start=st, stop=sp)
                        nc.tensor.matmul(rsE[:], pim_l, sie_sb[:, fc, 0:HB],
                                         start=st, stop=sp)
                    for fc in range(5, FC):
                        st, sp = (fc == 5), (fc == FC - 1)
                        pre_l = pre[:, fc, bass.ts(cc, P)]
                        pim_l = pim[:, fc, bass.ts(cc, P)]
                        nc.tensor.matmul(rcO[:], pre_l, cie_sb[:, fc, 0:HB],
                                         start=st, stop=sp)
                        nc.tensor.matmul(rsO[:], pim_l, sie_sb[:, fc, 0:HB],
                                         start=st, stop=sp)
                        nc.tensor.matmul(rsO2[:], pim_l, sie_sb[:, fc, HB:HB + P],
                                         start=st, stop=sp)
                    rcO_sb = ctmp.tile([P, HB], dt.float32, tag="rcO_sb")
                    nc.scalar.activation(rcO_sb[:], rcO[:], AF.Copy)
                    rsE_sb = ctmp.tile([P, HB], dt.float32, tag="rsE_sb")
                    nc.scalar.activation(rsE_sb[:], rsE[:], AF.Copy)
                    rsO_sb = ctmp.tile([P, HB + 1], dt.float32, tag="rsO_sb")
                    nc.scalar.activation(rsO_sb[:, 0:HB], rsO[:], AF.Copy)
                    nc.scalar.activation(rsO_sb[:, HB:HB + 1], rsO2[:, 0:1], AF.Copy)
                    s1 = ctmp.tile([P, HB], dt.float32, tag="s1")
                    nc.vector.tensor_tensor(out=s1[:], in0=rcE[:], in1=rcO_sb[:],
                                            op=OP.add)
                    s2 = ctmp.tile([P, HB], dt.float32, tag="s2")
                    nc.vector.tensor_tensor(out=s2[:], in0=rcE[:], in1=rcO_sb[:],
                                            op=OP.subtract)
                    w1 = ctmp.tile([P, HB], dt.float32, tag="w1")
                    nc.vector.tensor_tensor(out=w1[:], in0=rsE_sb[:],
                                            in1=rsO_sb[:, 0:HB], op=OP.add)
                    w2 = ctmp.tile([P, HB], dt.float32, tag="w2")
                    nc.vector.tensor_tensor(out=w2[:], in0=rsO_sb[:, 0:HB],
                                            in1=rsE_sb[:], op=OP.subtract)
                    rt = r_sb[cc]
                    nc.vector.tensor_tensor(out=rt[:, 0:HB], in0=s1[:], in1=w1[:],
                                            op=OP.add)
                    nc.vector.tensor_tensor(out=rt[:, 1023:HB:-1], in0=s2[:, 1:HB],
                                            in1=w2[:, 1:HB], op=OP.add)
                    nc.vector.tensor_tensor(out=rt[:, 1025:1536], in0=s2[:, 1:HB],
                                            in1=w2[:, 1:HB], op=OP.subtract)
                    nc.vector.tensor_tensor(out=rt[:, T - 1:1536:-1], in0=s1[:, 1:HB],
                                            in1=w1[:, 1:HB], op=OP.subtract)
                    nc.vector.tensor_tensor(out=rt[:, HB:HB + 1], in0=rcE2[:, 0:1],
                                            in1=rsO_sb[:, HB:HB + 1], op=OP.add)
                    nc.vector.tensor_tensor(out=rt[:, H:H + 1], in0=rcE[:, 0:1],
                                            in1=rcO_sb[:, 0:1], op=OP.subtract)
                    nc.vector.tensor_tensor(out=rt[:, 1536:1537], in0=rcE2[:, 0:1],
                                            in1=rsO_sb[:, HB:HB + 1], op=OP.subtract)
            es_p.close()  # pre/pim no longer needed

            # ---- D: top-k, softmax weights, gather ----
            agg_pool = es_agg.enter_context(tc.tile_pool(name=f"agg{b}", bufs=1, side="right"))
            agg = [agg_pool.tile([P, T], dt.float32r, tag=f"agg{j}", name=f"agg_sb{j}")
                   for j in range(NE)]
            with tc.tile_pool(name=f"d{b}", bufs=2) as dp:
                for cc in range(CC):
                    vals = dp.tile([P, 8], dt.float32, tag="vals")
                    idx = dp.tile([P, 8], dt.uint32, tag="idx")
                    nc.vector.max(vals[:], r_sb[cc][:])
                    nc.vector.max_index(idx[:], vals[:], r_sb[cc][:])
                    negm = dp.tile([P, 1], dt.float32, tag="negm")
                    nc.scalar.activation(negm[:], vals[:, 0:1],
                                         AF.Copy, bias=0.0, scale=-1.0)
                    esc = dp.tile([P, T], dt.float32, tag="esc")
                    s_col = dp.tile([P, 1], dt.float32, tag="s_col")
                    nc.scalar.activation(
                        esc[:], r_sb[cc][:], AF.Exp,
                        bias=negm[:, 0:1], scale=1.0,
                        accum_out=s_col[:, 0:1])
                    rs = dp.tile([P, 1], dt.float32, tag="rs")
                    nc.vector.reciprocal(rs[:], s_col[:])
                    ew = dp.tile([P, K], dt.float32, tag="ew")
                    nc.scalar.activation(ew[:], vals[:, 0:K],
                                         AF.Exp, bias=negm[:, 0:1],
                                         scale=1.0)
                    w3 = dp.tile([P, K], dt.float32, tag="w3")
                    nc.vector.tensor_scalar_mul(w3[:], ew[:], rs[:, 0:1])

                    # gather offsets: (b*512+cc*128+p)*4096 + 2048 - lag
                    iot = dp.tile([P, 1], dt.int32, tag="iot")
                    nc.gpsimd.iota(
                        iot[:], pattern=[[0, 1]],
                        base=(b * C + cc * P) * (2 * T) + T,
                        channel_multiplier=2 * T)
                    iot_f = dp.tile([P, 1], dt.float32, tag="iot_f")
                    nc.vector.tensor_copy(iot_f[:], iot[:])
                    idx_f = dp.tile([P, K], dt.float32, tag="idx_f")
                    nc.vector.tensor_copy(idx_f[:], idx[:, 0:K])
                    gof = dp.tile([P, K], dt.float32, tag="gof")
                    nc.scalar.activation(gof[:], idx_f[:],
                                         AF.Copy, bias=0.0, scale=-1.0)
                    nc.vector.tensor_scalar_add(gof[:], gof[:],
                                                iot_f[:, 0:1])
                    gou = dp.tile([P, K], dt.uint32, tag="gou")
                    nc.vector.tensor_copy(gou[:], gof[:])

                    for k in range(K):
                        a_t = agg[k * CC + cc]
                        for hh in range(2):
                            nc.gpsimd.indirect_dma_start(
                                out=a_t[:, bass.ts(hh, T // 2)], out_offset=None,
                                in_=v2[:, :],
                                in_offset=bass.IndirectOffsetOnAxis(
                                    ap=gou[:, k:k + 1], axis=1),
                                element_offset=hh * (T // 2))
                        nc.vector.tensor_scalar_mul(
                            a_t[:], a_t[:], w3[:, k:k + 1])
            es_r.close()  # r tiles no longer needed

            # ---- E: final projection ----
            with tc.tile_pool(name=f"e{b}", bufs=1) as ep, \
                 tc.tile_pool(name=f"eps{b}", bufs=8, space="PSUM") as eps:
                wf_sb = ep.tile([P, NE, C], dt.float32r, tag="wf_sb")
                nc.sync.dma_start(
                    wf_sb[:], Wf.rearrange("(n p) d -> p n d", p=P))
                for dc in range(CC):
                    for tb in range(4):
                        ps = eps.tile([P, T // 4], dt.float32,
                                      tag="out_ps")
                        for j in range(NE):
                            nc.tensor.matmul(
                                ps[:], wf_sb[:, j, bass.ts(dc, P)],
                                agg[j][:, bass.ts(tb, T // 4)],
                                start=(j == 0), stop=(j == NE - 1))
                        o_sb = ep.tile([P, T // 4], dt.float32,
                                       tag="o_sb")
                        nc.scalar.activation(o_sb[:], ps[:], AF.Copy)
                        nc.sync.dma_start(
                            out2[b, bass.ts(dc, P),
                                 bass.ts(tb, T // 4)], o_sb[:])
            es_agg.close()

    nc.compile()
    return nc


def _get_nc():
    if "nc" not in _CACHE:
        _CACHE["nc"] = _build()
    return _CACHE["nc"]


def kernel(query, key, value, Wq, bq, Wk, bk, Wv, bv, Wf, bf):
    query = np.ascontiguousarray(np.asarray(query, dtype=np.float32))
    key = np.ascontiguousarray(np.asarray(key, dtype=np.float32))
    value = np.ascontiguousarray(np.asarray(value, dtype=np.float32))
    for bias in (bq, bk, bv, bf):
        assert np.all(np.asarray(bias) == 0.0), "nonzero biases unsupported"

    if "mats" not in _CACHE:
        _CACHE["mats"] = _dft_matrices()
    wree, wreo, wime, wimo, cie, sie = _CACHE["mats"]

    shared = {
        "Wq": np.ascontiguousarray(np.asarray(Wq, np.float32)),
        "Wk": np.ascontiguousarray(np.asarray(Wk, np.float32)),
        "Wv": np.ascontiguousarray(np.asarray(Wv, np.float32)),
        "Wf": np.ascontiguousarray(np.asarray(Wf, np.float32)),
        "Wree": wree, "Wreo": wreo, "Wime": wime, "Wimo": wimo,
        "Cie": cie, "Sie": sie,
    }
    in_maps = []
    for c in range(NCORES):
        sl = slice(c * NB, (c + 1) * NB)
        in_maps.append({
            "query2": query[sl], "key2": key[sl], "value2": value[sl], **shared})

    from concourse.bass_utils import run_bass_kernel_spmd
    nc = _get_nc()
    res = run_bass_kernel_spmd(nc, in_maps, core_ids=list(range(NCORES)))
    _CACHE["last_results"] = res
    out = np.concatenate([res.results[c]["out2"] for c in range(NCORES)], axis=0)
    return out.astype(np.float32)

